# revision 47
# baseline (speedup 1.0000x reference)
"""Trainium2 Bass kernel for nn_CMFA (dense_transformer, seq_len=1 cross-attention).

Math notes (exact simplifications vs the reference):
  - softmax over a single key is exactly 1.0, so the attention output is
    exactly the v-projection: mha(q,k,v) = (v @ Wv.T + bv) @ Wo.T + bo.
    The q/k projections never influence the output.
  - Wv -> Wo -> fi2 is a linear chain (no nonlinearity), so it is folded on
    the host:  V = [v1, i_] @ Wcat.T + bcat  with
      Wcat = [fi2 @ (Wo @ Wv), fi2],  bcat = fi2 @ (Wo @ bv + bo) + fi2_b
    (the i_ column block carries the residual through fi2).

Precision strategy (driven by an exact-input host simulation that has
matched hardware to 4 significant digits at every step):
  - Quantization noise injected at the v1/v2 level reaches the output only
    through the folded small-weight Wvo chain, so it is strongly
    attenuated; noise on the i_/t_ residual path lands at full strength.
  - Therefore: ci1/ct1 run entirely in fp8e4 DoubleRow (2 contraction
    chunks per matmul, ~2x measured); the cat layers' v-halves run in
    DoubleRow (v1/v2 activations are written as fp8 directly) while their
    residual halves stay fp16; fi1 runs its last contraction chunk-pair in
    DoubleRow and the remaining 14 chunks in fp16.  The cat/fi1 mixed
    PSUM groups stay uniform by scaling BOTH halves' weights by 2^10
    (exact in fp16) and undoing it in the activation's scale parameter.
  - Everything else (inputs, fi1/ft1 weights, i_/t_ activations) is fp16;
    PSUM accumulation and biases are f32; output stores are f16.
  - Simulated end-to-end error 1.28e-2 vs the 2e-2 gate.  Full-fp8
    variants fail: e4m3 on all of fi1 or on the cat residual half measures
    3.7-5.1e-2.

Device layout: activations are feature-major ("transposed", [feat, batch]) so
every matmul contracts over the partition dim and every DMA is contiguous.
The host pre-transposes the batch shards of i/t and transposes the output
back. Pure data parallel across 8 cores; weights replicated.

Schedule (trace-measured):
  - PE-bound; the matmul stream runs gap-free (fp16 512-wide at 213ns,
    DoubleRow at ~215ns for twice the contraction).
  - The framework preamble (engine barriers + TENSOR_LOADs) runs to ~7us;
    nothing can issue before it.  The two loads that gate the first real
    layer (wt1 and x0's t-chunk) go one per HWDGE ring (sync/scalar) so
    their startup transfers overlap; a single N=128 warm-up matmul on wt1
    covers the residual latency and opens the HAM busy window.
  - 8 cores share HBM, so the startup flood is rationed: the preamble
    issues ONLY tile-0's t_/i_ needs; wc/wV/wT/x-tile-1 issue from the
    scalar queue BEHIND act instructions, which gates their transfer on
    pipeline progress.  Later x tiles prefetch just-in-time via the
    bufs=2 x-pool WAR dependency.
  - Per tile: t_ first (tiny gate), then fi1 k-outer (4 matmuls per
    arriving chunk, DR pair last so its data has the longest lead), then
    ct1/ci1 in DR, then the PREVIOUS tile's folded output layers (one-tile
    software pipeline keeps wV/wT deadlines late).
  - Output stores are f16 on the sync queue; the kernel's closing block
    computes in two half-column PSUM groups with acts split over
    scalar+vector and stores over both HWDGE rings.
"""

import numpy as np

B, IMG, TAB, HID = 32768, 2048, 128, 512
NCORES = 8
BS = B // NCORES  # rows per core
NT = 512          # batch-tile (matmul moving/free dim)
KI = IMG // 128   # 16 contraction chunks for fi1
FI1_PAIRS = 1     # fi1 chunk-pairs run in fp8 DoubleRow (error budget caps at 1)
KF16 = KI - 2 * FI1_PAIRS  # fp16 fi1 chunks
XALL = KF16 + 1   # x tile: t chunk + fp16 i chunks
NWARM = 1         # PE p-state warm-up matmuls (on the wt1 tile, earliest load)
WS = 1024.0       # pow2 weight scale for fp8 layers (keeps e4m3 normal-range;
                  # applied to BOTH halves of mixed groups, undone in the act)

_CACHE = {}


def _pack_blocks(WT: np.ndarray, K: int, M: int, scale: float = 1.0) -> np.ndarray:
    """[K*128, M*128] -> [128, K, M*128] with [p, k, m*128+j] = WT[k*128+p, m*128+j]."""
    out = WT.reshape(K, 128, M * 128).transpose(1, 0, 2).astype(np.float64) * scale
    return np.ascontiguousarray(out.astype(np.float32)).astype(np.float16)


def _pack_blocks8(WT: np.ndarray, K: int, M: int, scale: float) -> np.ndarray:
    """Same block layout, scaled by a power of 2 and cast to TRN fp8e4
    (ml_dtypes.float8_e4m3: max +-240, RNE)."""
    import ml_dtypes
    out = WT.reshape(K, 128, M * 128).transpose(1, 0, 2).astype(np.float64) * scale
    return np.ascontiguousarray(out.astype(np.float32)).astype(ml_dtypes.float8_e4m3)


def _build_nc(bs: int):
    import concourse.bass as bass
    import concourse.tile as tile
    from concourse import bacc, mybir

    f32 = mybir.dt.float32
    f16 = mybir.dt.float16
    f8 = mybir.dt.float8e4
    DR = mybir.MatmulPerfMode.DoubleRow
    Relu = mybir.ActivationFunctionType.Relu
    Ident = mybir.ActivationFunctionType.Identity
    Mult = mybir.AluOpType.mult
    Add = mybir.AluOpType.add
    ntiles = bs // NT

    nc = bacc.Bacc("TRN2", target_bir_lowering=False, debug=False)

    # tile-major input layout: per batch-tile, chunk 0 = t, chunks 1..14 =
    # fp16 i chunks; the last fi1 chunk-pair ships separately as fp8
    iT_d = nc.dram_tensor("iT", [ntiles, 128, XALL, NT], f16,
                          kind="ExternalInput").ap()
    iT8_d = nc.dram_tensor("iT8", [ntiles, 128, 2 * FI1_PAIRS, NT], f8,
                           kind="ExternalInput").ap()
    w_fi1_d = nc.dram_tensor("w_fi1", [128, KF16, 512], f16, kind="ExternalInput").ap()
    w_fi18_d = nc.dram_tensor("w_fi18", [128, 2 * FI1_PAIRS, 512], f8,
                              kind="ExternalInput").ap()
    w_ft1_d = nc.dram_tensor("w_ft1", [128, 1, 512], f16, kind="ExternalInput").ap()
    w_ci1_d = nc.dram_tensor("w_ci1", [128, 4, 512], f8, kind="ExternalInput").ap()
    w_ct1_d = nc.dram_tensor("w_ct1", [128, 4, 512], f8, kind="ExternalInput").ap()
    # cat weights: fp8 v-half + fp16 residual half (both x1024)
    w_V8_d = nc.dram_tensor("w_V8", [128, 4, 512], f8, kind="ExternalInput").ap()
    w_T8_d = nc.dram_tensor("w_T8", [128, 4, 512], f8, kind="ExternalInput").ap()
    w_V_d = nc.dram_tensor("w_V", [128, 4, 512], f16, kind="ExternalInput").ap()
    w_T_d = nc.dram_tensor("w_T", [128, 4, 512], f16, kind="ExternalInput").ap()
    bias_d = nc.dram_tensor("bias", [128, 24], f32, kind="ExternalInput").ap()
    # f16 output: halves store traffic and the closing transfer
    out_d = nc.dram_tensor("outT", [2 * HID, bs], f16, kind="ExternalOutput").ap()

    # fi1 chunk groups for tile 0 (fp16 i-chunk indices): group completion
    # is all-or-nothing, so the FIRST groups are single chunks; the back
    # groups are coarse since they arrive far ahead of consumption.
    WGRP = [(0, 1), (1, 2), (2, 4), (4, 6), (6, 10), (10, 14)]

    with tile.TileContext(nc) as tc:
        with (
            tc.tile_pool(name="w", bufs=1) as wpool,
            tc.tile_pool(name="x", bufs=2) as xpool,
            tc.tile_pool(name="h", bufs=8) as hpool,
            tc.tile_pool(name="q8", bufs=2) as q8pool,
            tc.tile_pool(name="o", bufs=8) as opool,
            tc.tile_pool(name="ps", bufs=8, space="PSUM") as pspool,
        ):
            wf1 = wpool.tile([128, KF16, 512], f16, name="w_fi1_t")
            wf18 = wpool.tile([128, 2 * FI1_PAIRS, 512], f8, name="w_fi18_t")
            wt1 = wpool.tile([128, 1, 512], f16, name="w_ft1_t")
            wc1 = wpool.tile([128, 4, 512], f8, name="w_ci1_t")
            wc2 = wpool.tile([128, 4, 512], f8, name="w_ct1_t")
            wV8 = wpool.tile([128, 4, 512], f8, name="w_V8_t")
            wT8 = wpool.tile([128, 4, 512], f8, name="w_T8_t")
            wV = wpool.tile([128, 4, 512], f16, name="w_V_t")
            wT = wpool.tile([128, 4, 512], f16, name="w_T_t")
            bt = wpool.tile([128, 24], f32, name="bias_t")

            # ---- preamble loads: ONLY what tile 0's t_/i_ layers need.
            # 8 cores flood the shared HBM at startup, so every byte issued
            # here delays the startup-critical chunks on every core.  All
            # later-needed tensors issue from the scalar queue BEHIND act
            # instructions inside the n=0 body (FIFO gating on progress).
            # wt1 and x0's t-chunk gate the first real layer (t_): one on
            # each HWDGE ring so their startup transfers overlap.
            x_cur = xpool.tile([128, XALL, NT], f16, tag="x", name="x_0")
            x_1 = xpool.tile([128, XALL, NT], f16, tag="x", name="x_1")
            nc.sync.dma_start(wt1[:], w_ft1_d[:])
            nc.scalar.dma_start(x_cur[:, 0, :], iT_d[0, :, 0, :])
            x8_0 = q8pool.tile([128, 2 * FI1_PAIRS, NT], f8, tag="x8", name="x8_0")
            for gi, (a, b) in enumerate(WGRP):
                nc.sync.dma_start(x_cur[:, a + 1:b + 1, :], iT_d[0, :, a + 1:b + 1, :])
                nc.scalar.dma_start(wf1[:, a:b, :], w_fi1_d[:, a:b, :])
                if gi == 0:
                    # bias is only needed by the t_ act (~12us); k0's weight
                    # group outranks it on the scalar ring
                    nc.scalar.dma_start(bt[:], bias_d[:])
                    # the small fp8 fi1 pair rides just behind the k0 gate:
                    # it is the SECOND accumulation step, so its early
                    # arrival turns the tile-0 chunk-delivery stall into
                    # real work
                    nc.sync.dma_start(x8_0[:], iT8_d[0])
                    nc.scalar.dma_start(wf18[:], w_fi18_d[:])

            # ---- PE warm-up on wt1 (N=128: cheap): starts the HAM busy
            # window early and bridges the t-chunk/bias DMA latency ----
            if NWARM:
                wps = pspool.tile([128, NT], f32, tag="ps", name="warm_ps")
                for _ in range(NWARM):
                    nc.tensor.matmul(wps[:, 0:128], wt1[:, 0, 0:128],
                                     wt1[:, 0, 0:128], start=True, stop=True)

            def act(ps, htag, n, m, bcol, func, scale=1.0):
                h = hpool.tile([128, NT], f16, tag=htag, name=f"{htag}_{n}_{m}")
                nc.scalar.activation(h[:], ps[:], func, scale=scale,
                                     bias=bt[:, bcol + m:bcol + m + 1])
                return h

            def fi1_layer(xs, x8, n):
                """i_ = relu((x @ fi1.T)/WS + b): 14 fp16 chunks (k outer,
                rate-matches chunked DMA arrival) + 1 fp8 DoubleRow pair.
                The pair accumulates right after k0: its 256KB of fp8 data
                arrives early, so during tile 0 it fills the chunk-delivery
                stall instead of waiting at the end."""
                ps = [pspool.tile([128, NT], f32, tag="ps", name=f"ps_i__{n}_{m}")
                      for m in range(4)]
                for k in range(KF16):
                    for m in range(4):
                        nc.tensor.matmul(ps[m][:], wf1[:, k, m * 128:(m + 1) * 128],
                                         xs[k], start=(k == 0), stop=(k == KF16 - 1))
                    if k == 0:
                        for m in range(4):
                            nc.tensor.matmul(ps[m][:],
                                             wf18[:, 0:2, m * 128:(m + 1) * 128],
                                             x8[:, 0:2, :], start=False, stop=False,
                                             perf_mode=DR)
                return [act(ps[m], "i_", n, m, 0, Relu, scale=1.0 / WS)
                        for m in range(4)]

            def layer_m_outer(wt, xs, htag, n, bcol, K):
                """m outer: each PSUM bank closes after its k loop and drains
                on the scalar engine while the PE works on the next block."""
                outs = []
                for m in range(4):
                    ps = pspool.tile([128, NT], f32, tag="ps", name=f"ps_{htag}_{n}_{m}")
                    for k in range(K):
                        nc.tensor.matmul(ps[:], wt[:, k, m * 128:(m + 1) * 128],
                                         xs[k], start=(k == 0), stop=(k == K - 1))
                    outs.append(act(ps, htag, n, m, bcol, Relu))
                return outs

            def layer_dr(wt8, x8, v8out, n, bcol):
                """fp8 DoubleRow 512-contraction layer (ci1/ct1): 2 matmuls
                per output block, each contracting 2 chunks.  The act undoes
                the pow2 weight scale and writes fp8 directly (consumed only
                by the cat v-half, which is noise-attenuated)."""
                for m in range(4):
                    ps = pspool.tile([128, NT], f32, tag="ps", name=f"ps_v{bcol}_{n}_{m}")
                    for g in range(2):
                        nc.tensor.matmul(ps[:], wt8[:, 2 * g:2 * g + 2,
                                                    m * 128:(m + 1) * 128],
                                         x8[:, 2 * g:2 * g + 2, :],
                                         start=(g == 0), stop=(g == 1),
                                         perf_mode=DR)
                    nc.scalar.activation(v8out[:, m, :], ps[:], Relu,
                                         scale=1.0 / WS,
                                         bias=bt[:, bcol + m:bcol + m + 1])

            def cat_layer(w8, w16, v8, ts_b, n, bcol, oname, orow0, final=False):
                """out[m] = (v8 @ w8[m] + resid @ w16[m])/WS + bias; f16 store.
                v-half: 2 DoubleRow matmuls on the fp8 v1/v2; residual half:
                4 fp16 matmuls (precision-critical).  Bias+scale alternates
                scalar/vector so blocks drain in parallel; stores ride sync."""
                def mms(pso, m, cs):
                    for g in range(2):
                        nc.tensor.matmul(pso[:], w8[:, 2 * g:2 * g + 2,
                                                    m * 128:(m + 1) * 128],
                                         v8[:, 2 * g:2 * g + 2, cs],
                                         start=(g == 0), stop=False,
                                         perf_mode=DR)
                    for k in range(4):
                        nc.tensor.matmul(pso[:], w16[:, k, m * 128:(m + 1) * 128],
                                         ts_b[k][:, cs], start=False, stop=(k == 3))

                full = slice(0, NT)
                for m in range(4):
                    o = opool.tile([128, NT], f16, tag="o", name=f"o{oname}_{n}_{m}")
                    bias_ap = bt[:, bcol + m:bcol + m + 1]
                    rows = out_d[orow0 + 128 * m:orow0 + 128 * (m + 1),
                                 n * NT:(n + 1) * NT]
                    if final and m == 3:
                        # closing chain: two half-column PSUM groups so half
                        # 0's act+store issue while half 1's matmuls run
                        H = NT // 2
                        for h, (eng_act, eng_dma) in enumerate(
                                ((nc.scalar, nc.sync), (nc.vector, nc.scalar))):
                            ph = pspool.tile([128, H], f32, tag="ps",
                                             name=f"ps_{oname}_{n}_3{h}")
                            cs = slice(h * H, h * H + H)
                            mms(ph, m, cs)
                            if eng_act is nc.scalar:
                                nc.scalar.activation(o[:, cs], ph[:], Ident,
                                                     scale=1.0 / WS, bias=bias_ap)
                            else:
                                nc.vector.tensor_scalar(o[:, cs], ph[:], 1.0 / WS,
                                                        bias_ap, op0=Mult, op1=Add)
                            eng_dma.dma_start(rows[:, cs], o[:, cs])
                        continue
                    ps = pspool.tile([128, NT], f32, tag="ps", name=f"ps_{oname}_{n}_{m}")
                    mms(ps, m, full)
                    if m % 2 == 0:
                        nc.scalar.activation(o[:], ps[:], Ident,
                                             scale=1.0 / WS, bias=bias_ap)
                    else:
                        nc.vector.tensor_scalar(o[:], ps[:], 1.0 / WS, bias_ap,
                                                op0=Mult, op1=Add)
                    nc.sync.dma_start(rows, o[:])

            def vt_phase(n, v18, i_, v28, t_, final=False):
                # ---- V = [v1, i_] @ WcatV.T + bcatV ----
                cat_layer(wV8, wV, v18, i_, n, 16, "V", 0)
                # ---- T = [v2, t_] @ WcatT.T + bcatT ----
                cat_layer(wT8, wT, v28, t_, n, 20, "T", HID, final=final)

            xtiles = [x_cur, x_1]
            x8tiles = [x8_0]
            prev = None
            for n in range(ntiles):
                x_n = xtiles[n]
                xs_i = [x_n[:, k + 1, :] for k in range(KF16)]

                # JIT prefetch of tile n+1 (x pool bufs=2: the issue WARs the
                # tile n-1 slot, so the transfer starts right as tile n does
                # -- a full tile-time ahead of need, and never during the
                # 8-core startup HBM crunch)
                if n >= 1 and n + 1 < ntiles:
                    x_nxt = xpool.tile([128, XALL, NT], f16, tag="x", name=f"x_{n + 1}")
                    nc.sync.dma_start(x_nxt[:, 0:8, :], iT_d[n + 1, :, 0:8, :])
                    nc.sync.dma_start(x_nxt[:, 8:XALL, :], iT_d[n + 1, :, 8:XALL, :])
                    xtiles.append(x_nxt)
                if n + 1 < ntiles:
                    x8_nxt = q8pool.tile([128, 2 * FI1_PAIRS, NT], f8, tag="x8",
                                         name=f"x8_{n + 1}")
                    nc.sync.dma_start(x8_nxt[:], iT8_d[n + 1])
                    x8tiles.append(x8_nxt)

                # ---- t_ = relu(t @ ft1.T + b): gates on only 256KB of input ----
                t_ = layer_m_outer(wt1, [x_n[:, 0, :]], "t_", n, 4, 1)
                if n == 0:
                    # wc loads ride the scalar queue behind the t_ acts:
                    # transfer starts once tile 0 is underway, lands well
                    # before the v2/v1 layers need them
                    nc.scalar.dma_start(wc2[:], w_ct1_d[:])
                    nc.scalar.dma_start(wc1[:], w_ci1_d[:])
                # fp8 copies of t_ for the ct1 DoubleRow moving operand (the
                # f16 originals still feed the catT residual block).  DVE
                # converts; it is idle here.
                t8 = q8pool.tile([128, 4, NT], f8, tag="t8", name=f"t8_{n}")
                for m in range(4):
                    nc.vector.tensor_scalar_add(t8[:, m, :], t_[m][:], 0.0)
                # ---- i_ = relu(i @ fi1.T + b) ----
                i_ = fi1_layer(xs_i, x8tiles[n], n)
                i8 = q8pool.tile([128, 4, NT], f8, tag="i8", name=f"i8_{n}")
                for m in range(4):
                    nc.vector.tensor_scalar_add(i8[:, m, :], i_[m][:], 0.0)
                if n == 0:
                    # x tile 1 behind the i_ acts (needed from ~tile 1 mid)
                    nc.scalar.dma_start(x_1[:, 0:8, :], iT_d[1, :, 0:8, :])
                    nc.scalar.dma_start(x_1[:, 8:XALL, :], iT_d[1, :, 8:XALL, :])

                # ---- v2 = relu(t_ @ ct1.T + b), v1 = relu(i_ @ ci1.T + b):
                # fp8 DoubleRow, written as fp8 for the cat v-halves ----
                v28 = q8pool.tile([128, 4, NT], f8, tag="v28", name=f"v28_{n}")
                layer_dr(wc2, t8, v28, n, 12)
                v18 = q8pool.tile([128, 4, NT], f8, tag="v18", name=f"v18_{n}")
                layer_dr(wc1, i8, v18, n, 8)
                if n == 0:
                    # cat weights behind the v acts (needed from the deferred
                    # vt_phase(0), which runs inside tile 1)
                    nc.scalar.dma_start(wV8[:], w_V8_d[:])
                    nc.scalar.dma_start(wV[:], w_V_d[:])
                    nc.scalar.dma_start(wT8[:], w_T8_d[:])
                    nc.scalar.dma_start(wT[:], w_T_d[:])

                # ---- V/T of the PREVIOUS tile (one-tile software pipeline:
                # moves the wV/wT load deadlines out of the startup crunch) ----
                if prev is not None:
                    vt_phase(n - 1, *prev)
                prev = (v18, i_, v28, t_)

            vt_phase(ntiles - 1, *prev, final=True)

    nc.compile()
    return nc


def _host_pack(inp: dict):
    import ml_dtypes
    f8 = np.float64
    fi1_w, fi1_b = inp["fi1_w"], inp["fi1_b"]
    ft1_w, ft1_b = inp["ft1_w"], inp["ft1_b"]
    ci1_w, ci1_b = inp["ci1_w"], inp["ci1_b"]
    ct1_w, ct1_b = inp["ct1_w"], inp["ct1_b"]

    def fold(wv, bv, wo, bo, f_w, f_b):
        Wvo = wo.astype(f8) @ wv.astype(f8)
        bvo = wo.astype(f8) @ bv.astype(f8) + bo.astype(f8)
        Wcat = np.concatenate([f_w.astype(f8) @ Wvo, f_w.astype(f8)], axis=1)
        bcat = f_w.astype(f8) @ bvo + f_b.astype(f8)
        return Wcat.astype(np.float32), bcat.astype(np.float32)

    WcatV, bcatV = fold(inp["aV_wv"], inp["aV_bv"], inp["aV_wo"], inp["aV_bo"],
                        inp["fi2_w"], inp["fi2_b"])
    WcatT, bcatT = fold(inp["aT_wv"], inp["aT_bv"], inp["aT_wo"], inp["aT_bo"],
                        inp["ft2_w"], inp["ft2_b"])

    fi1T = np.ascontiguousarray(fi1_w.T)         # [2048, 512]
    kcut = 128 * KF16
    WVT = np.ascontiguousarray(WcatV.T)          # [1024, 512]
    WTT = np.ascontiguousarray(WcatT.T)
    weights = {
        # fi1 fp16 part and fp8 pair are both x WS (exact in fp16: pow2)
        "w_fi1": _pack_blocks(fi1T[:kcut], KF16, 4, WS),
        "w_fi18": _pack_blocks8(fi1T[kcut:], 2 * FI1_PAIRS, 4, WS),
        "w_ft1": _pack_blocks(np.ascontiguousarray(ft1_w.T), 1, 4),
        "w_ci1": _pack_blocks8(np.ascontiguousarray(ci1_w.T), 4, 4, WS),
        "w_ct1": _pack_blocks8(np.ascontiguousarray(ct1_w.T), 4, 4, WS),
        "w_V8": _pack_blocks8(WVT[:HID], 4, 4, WS),
        "w_V": _pack_blocks(WVT[HID:], 4, 4, WS),
        "w_T8": _pack_blocks8(WTT[:HID], 4, 4, WS),
        "w_T": _pack_blocks(WTT[HID:], 4, 4, WS),
    }
    cols = []
    for b in (fi1_b, ft1_b, ci1_b, ct1_b, bcatV, bcatT):
        for m in range(4):
            cols.append(b[128 * m:128 * (m + 1)])
    weights["bias"] = np.ascontiguousarray(np.stack(cols, axis=1), dtype=np.float32)
    return weights


def make_in_maps(inputs: dict):
    """Full inputs -> per-core input dicts (shard batch, replicate weights)."""
    import ml_dtypes
    inputs = {k: np.asarray(v) for k, v in inputs.items()}
    i = np.asarray(inputs["i"], dtype=np.float32)
    t = np.asarray(inputs["t"], dtype=np.float32)
    weights = _host_pack(inputs)
    kcut = 128 * KF16
    i16 = i[:, :kcut].astype(np.float16)
    i8v = i[:, kcut:].astype(ml_dtypes.float8_e4m3)   # single-round from f32
    t16 = t.astype(np.float16)
    ntiles = BS // NT
    in_maps = []
    for c in range(NCORES):
        sl = slice(c * BS, (c + 1) * BS)
        m = dict(weights)
        # iT [ntiles, 128, XALL, NT]: chunk 0 = t, chunk 1+k = fp16 i chunk k
        xi = i16[sl].T.reshape(KF16, 128, ntiles, NT)   # [k, p, n, j]
        xt = t16[sl].T.reshape(TAB, ntiles, NT)         # [p, n, j]
        full = np.empty((ntiles, 128, XALL, NT), dtype=np.float16)
        full[:, :, 0, :] = xt.transpose(1, 0, 2)
        full[:, :, 1:, :] = xi.transpose(2, 1, 0, 3)
        m["iT"] = full
        # iT8 [ntiles, 128, 2, NT]: the fp8 fi1 chunk-pair
        x8 = i8v[sl].T.reshape(2 * FI1_PAIRS, 128, ntiles, NT)
        m["iT8"] = np.ascontiguousarray(x8.transpose(2, 1, 0, 3))
        in_maps.append(m)
    return in_maps


def kernel(**inputs) -> np.ndarray:
    from concourse import bass_utils

    if "nc" not in _CACHE:
        _CACHE["nc"] = _build_nc(BS)
    nc = _CACHE["nc"]

    in_maps = make_in_maps(inputs)
    res = bass_utils.run_bass_kernel_spmd(nc, in_maps, core_ids=list(range(NCORES)))

    out = np.empty((B, 2 * HID), dtype=np.float32)
    for c in range(NCORES):
        out[c * BS:(c + 1) * BS] = res.results[c]["outT"].T.astype(np.float32)
    return out
